# revision 18
# baseline (speedup 1.0000x reference)
"""Causal attention kernel for Trainium2, 8 NeuronCores.

Problem: x[4,2048,2048] @ Wq/Wk/Wv[2048,2048] -> causal softmax attention.

Sharding: 2 cores per batch; each core owns 1024 query rows, assigned as
global 512-row chunks {0,3} (even cores) / {1,2} (odd cores) to balance causal
work. Each core computes Q^T, K^T and V only for its OWN rows (its x^T input
holds just those 1024 columns), then a pairwise AllGather assembles the full
K^T / V for the batch. Gathered key blocks land in pair-rank order
[chunk0, chunk3, chunk1, chunk2], identical on every core, so the SPMD
program is uniform: query slot 0 (chunk c_lo) attends to key positions
{0-3, 8-11}, slot 1 (chunk c_hi) to all 16; true causality for the permuted
key order is enforced by per-core mask tensors (input data).

On-device layout: scores are computed transposed (S^T = K Q^T, keys on
partitions) so exp(S^T) feeds the attention@V matmul directly as the
stationary operand with no transpose; row sums come from a matmul against a
ones vector; softmax max-subtraction is skipped (softmax is shift-invariant;
scores are O(1) here so fp32 range is ample).

dtypes: projections + scores in float32r (full PE rate, ~1e-4 rounding),
probabilities/V in bf16 for the AV matmul, all accumulation fp32.
"""

import math

import numpy as np
import ml_dtypes

import concourse.bass as bass
import concourse.mybir as mybir
import concourse.tile as tile
from concourse import bacc
from concourse.bass import ds, ts
from concourse.bass_utils import run_bass_kernel_spmd

B, S, D = 4, 2048, 2048
P = 128
DC = D // P          # 16 contraction chunks
SB = S // P          # 16 key blocks
QROWS = 1024         # query rows per core
NCORES = 8
INV_SQRT_D = 1.0 / math.sqrt(D)

# gathered key-block position -> true 512-chunk (pair-rank order, all cores)
POS2TRUE = [0, 3, 1, 2]
# key-block positions processed by query slot 0
SLOT0_POS = [0, 1, 2, 3, 8, 9, 10, 11]
S0IDX = {pos: j for j, pos in enumerate(SLOT0_POS)}
PAIRS = [[0, 1], [2, 3], [4, 5], [6, 7]]

F32 = mybir.dt.float32
F32R = mybir.dt.float32r
BF16 = mybir.dt.bfloat16
XW_DT = BF16    # x^T and W inputs + projection matmuls
QK_DT = BF16    # Q^T/K^T staging + score matmuls
Exp = mybir.ActivationFunctionType.Exp

_CACHED_NC = None


def build_nc():
    global _CACHED_NC
    if _CACHED_NC is not None:
        return _CACHED_NC
    nc = bacc.Bacc(trn_type="TRN2", target_bir_lowering=False, debug=False,
                   num_devices=NCORES)

    xt_d = nc.dram_tensor("xt", [D, QROWS], XW_DT, kind="ExternalInput")
    wq_d = nc.dram_tensor("wq", [DC, P, DC, P], XW_DT, kind="ExternalInput")
    wk_d = nc.dram_tensor("wk", [DC, P, DC, P], XW_DT, kind="ExternalInput")
    wv_d = nc.dram_tensor("wv", [4, 2, P, 8, 512], XW_DT, kind="ExternalInput")
    mk_d = nc.dram_tensor("masks", [P, 24, 512], BF16, kind="ExternalInput")
    out_d = nc.dram_tensor("out", [QROWS, D], F32, kind="ExternalOutput")

    with tile.TileContext(nc) as tc:
        with tc.tile_pool(name="dram", bufs=1, space="DRAM") as dpool:
            qT = dpool.tile([P, DC, QROWS], QK_DT, tag="qT")      # [p, dc, q]
            kT_own = dpool.tile([8, P, DC, P], QK_DT, tag="kTo")  # [kb, p, dc, k]
            vv_own = dpool.tile([8, P, D], BF16, tag="vvo")      # [kb, p, d]
            kT = dpool.tile([2, 8, P, DC, P], QK_DT, tag="kT")    # gathered
            vv = dpool.tile([2, 8, P, D], BF16, tag="vv")        # gathered

            # ---------------- phase 1: projections ----------------
            with (
                tc.tile_pool(name="xt", bufs=1) as xt_pool,
                tc.tile_pool(name="wqk", bufs=16) as wqk_pool,
                tc.tile_pool(name="st", bufs=6) as st_pool,
                tc.tile_pool(name="ps1", bufs=8, space="PSUM") as ps1,
            ):
                # whole-phase W preloads (bf16 W is small enough to keep
                # an entire projection's weights resident); first few W tiles
                # and x^T go first so the K matmuls can start early
                def load_w(pool, dram, m, name):
                    wt = pool.tile([P, DC, P], XW_DT, tag="w", name=name)
                    nc.sync.dma_start(wt[:, :8, :], dram.ap()[m][:, :8, :])
                    nc.sync.dma_start(wt[:, 8:, :], dram.ap()[m][:, 8:, :])
                    return wt

                wk_pre = [load_w(wqk_pool, wk_d, m, f"wkp{m}") for m in range(4)]

                xts = [xt_pool.tile([P, DC, 512], XW_DT, tag=f"xt{c}",
                                    name=f"xt{c}")
                       for c in range(2)]
                for c in range(2):
                    for dc in range(DC):
                        nc.sync.dma_start(
                            xts[c][:, dc, :],
                            xt_d.ap()[ds(dc * P, P), ts(c, 512)])
                wk_pre += [load_w(wqk_pool, wk_d, m, f"wkp{m}")
                           for m in range(4, DC)]

                # --- K^T (own rows only)
                for m in range(DC):
                    wt = wk_pre[m]
                    for s in range(2):
                        ps = ps1.tile([P, 512], F32, tag="ps")
                        for dc in range(DC):
                            nc.tensor.matmul(
                                ps[:], lhsT=wt[:, dc, :],
                                rhs=xts[s][:, dc, :],
                                start=(dc == 0), stop=(dc == DC - 1),
                            )
                        st = st_pool.tile([P, 512], QK_DT, tag="st")
                        nc.scalar.copy(st[:], ps[:])
                        for j in range(4):
                            nc.sync.dma_start(kT_own[s * 4 + j, :, m, :],
                                              st[:, ts(j, P)])

                # gather K^T halves within each pair (overlaps V compute)
                nc.gpsimd.collective_compute(
                    "AllGather", mybir.AluOpType.bypass,
                    replica_groups=PAIRS,
                    ins=[kT_own.opt()], outs=[kT.opt()],
                )


                # --- Q^T (overlaps the K^T gather)
                wq_pre = [load_w(wqk_pool, wq_d, m, f"wq{m}")
                          for m in range(DC)]
                for m in range(DC):
                    wt = wq_pre[m]
                    for s in range(2):
                        ps = ps1.tile([P, 512], F32, tag="ps")
                        for dc in range(DC):
                            nc.tensor.matmul(
                                ps[:], lhsT=wt[:, dc, :],
                                rhs=xts[s][:, dc, :],
                                start=(dc == 0), stop=(dc == DC - 1),
                            )
                        st = st_pool.tile([P, 512], QK_DT, tag="st")
                        nc.scalar.copy(st[:], ps[:])
                        nc.sync.dma_start(qT[:, m, ts(s, 512)], st[:])

                # --- V (own rows only, natural layout, bf16)
                with (
                    tc.tile_pool(name="wv", bufs=8) as wv_pool,
                    tc.tile_pool(name="stv", bufs=4) as stv_pool,
                ):
                    wv_pre = []
                    for n in range(4):
                        for hb in range(2):
                            wvt = wv_pool.tile([P, 8, 512], XW_DT, tag="wv",
                                               name=f"wv{n}{hb}")
                            nc.sync.dma_start(wvt[:, :4, :],
                                              wv_d.ap()[n, hb][:, :4, :])
                            nc.sync.dma_start(wvt[:, 4:, :],
                                              wv_d.ap()[n, hb][:, 4:, :])
                            wv_pre.append(wvt)
                    for n in range(4):
                        wva = wv_pre[2 * n]
                        wvb = wv_pre[2 * n + 1]
                        for s in range(8):
                            ps = ps1.tile([P, 512], F32, tag="ps")
                            for dc in range(DC):
                                w = wva if dc < 8 else wvb
                                nc.tensor.matmul(
                                    ps[:], lhsT=xts[s // 4][:, dc, ts(s % 4, P)],
                                    rhs=w[:, dc % 8, :],
                                    start=(dc == 0), stop=(dc == DC - 1),
                                )
                            sv = stv_pool.tile([P, 512], BF16, tag="sv")
                            nc.vector.tensor_copy(sv[:], ps[:])
                            nc.sync.dma_start(vv_own[s, :, ts(n, 512)], sv[:])

                nc.gpsimd.collective_compute(
                    "AllGather", mybir.AluOpType.bypass,
                    replica_groups=PAIRS,
                    ins=[vv_own.opt()], outs=[vv.opt()],
                )

            # ---------------- phase 2: attention ----------------
            with (
                tc.tile_pool(name="qtv", bufs=2) as qtv_pool,
                tc.tile_pool(name="kt2", bufs=12) as kt_pool,
                tc.tile_pool(name="pt", bufs=1) as pt_pool,
                tc.tile_pool(name="mk", bufs=1) as mk_pool,
                tc.tile_pool(name="one", bufs=1) as one_pool,
                tc.tile_pool(name="sc", bufs=4) as sc_pool,
                tc.tile_pool(name="ob", bufs=4) as ob_pool,
                tc.tile_pool(name="ps_s", bufs=4, space="PSUM") as ps_s,
                tc.tile_pool(name="ps_av", bufs=3, space="PSUM") as ps_av,
                tc.tile_pool(name="ps_l", bufs=1, space="PSUM") as ps_l,
            ):
                mk = mk_pool.tile([P, 24, 512], BF16, tag="mk")
                for j in range(3):
                    nc.sync.dma_start(mk[:, ts(j, 8), :], mk_d.ap()[:, ts(j, 8), :])
                ones = one_pool.tile([P, 1], BF16, tag="ones")
                nc.vector.memset(ones[:], 1.0)
                # pt index: slot0 j -> key pos SLOT0_POS[j]; slot1 kb -> 8+kb
                pt = pt_pool.tile([P, 24, 512], BF16, tag="pt")

                # --- scores + exp.  qt/vt share pool slots (tag "qt"): the
                # two V super-tiles allocate into the slots the q tiles
                # release after their last score matmul.
                qts = []
                for slot in range(2):
                    qt = qtv_pool.tile([P, DC, 512], QK_DT, tag="qt",
                                       name=f"qt{slot}")
                    for j in range(4):
                        nc.sync.dma_start(qt[:, ts(j, 4), :],
                                          qT[:, ts(j, 4), ts(slot, 512)])
                    qts.append(qt)

                for kb in range(SB):
                    kt_t = kt_pool.tile([P, DC, P], QK_DT, tag="kt",
                                        name=f"kt{kb}")
                    src = kT[kb // 8, kb % 8]
                    for j in range(4):
                        nc.sync.dma_start(kt_t[:, ts(j, 4), :],
                                          src[:, ts(j, 4), :])
                    targets = []
                    if kb in S0IDX:
                        targets.append((S0IDX[kb], 0))
                    targets.append((8 + kb, 1))
                    for pti, slot in targets:
                        ps = ps_s.tile([P, 512], F32, tag="ps")
                        for dc in range(DC):
                            nc.tensor.matmul(
                                ps[:], lhsT=kt_t[:, dc, :],
                                rhs=qts[slot][:, dc, :],
                                start=(dc == 0), stop=(dc == DC - 1),
                            )
                        nc.scalar.activation(pt[:, pti, :], ps[:], Exp,
                                             scale=INV_SQRT_D)

                # --- V tiles reuse the qt slots; masks; AV; normalize
                vbig = []
                for half in range(2):
                    vb = qtv_pool.tile([P, 8, D], BF16, tag="qt",
                                       name=f"vb{half}")
                    for j in range(8):
                        nc.sync.dma_start(vb[:, j, :], vv[half, j])
                    vbig.append(vb)

                def vt_ap(pos):
                    return vbig[pos // 8][:, pos % 8, :]

                for j in range(24):
                    nc.vector.tensor_mul(pt[:, j, :], pt[:, j, :], mk[:, j, :])

                for slot in range(2):
                    if slot == 0:
                        idx = list(range(8))
                        kpos = SLOT0_POS
                    else:
                        idx = list(range(8, 24))
                        kpos = list(range(SB))
                    for qs in range(4):
                        pl = ps_l.tile([P, 1], F32, tag="pl")
                        for i, j in enumerate(idx):
                            nc.tensor.matmul(
                                pl[:], lhsT=pt[:, j, ts(qs, P)],
                                rhs=ones[:],
                                start=(i == 0), stop=(i == len(idx) - 1),
                            )
                        rl = sc_pool.tile([P, 1], F32, tag="rl")
                        nc.vector.reciprocal(rl[:], pl[:])
                        for n in range(4):
                            pav = ps_av.tile([P, 512], F32, tag="pav")
                            for i, j in enumerate(idx):
                                nc.tensor.matmul(
                                    pav[:], lhsT=pt[:, j, ts(qs, P)],
                                    rhs=vt_ap(kpos[i])[:, ts(n, 512)],
                                    start=(i == 0),
                                    stop=(i == len(idx) - 1),
                                )
                            ob = ob_pool.tile([P, 512], F32, tag="ob")
                            nc.vector.tensor_scalar_mul(ob[:], pav[:], rl[:])
                            nc.sync.dma_start(
                                out_d.ap()[ds(slot * 512 + qs * P, P),
                                           ts(n, 512)],
                                ob[:],
                            )

    nc.compile()
    _CACHED_NC = nc
    return nc


def _host_prep(x, Wq, Wk, Wv):
    """Build per-core input maps (host-side layout prep)."""
    np_xw = ml_dtypes.bfloat16 if XW_DT == BF16 else np.float32
    wq_h = np.ascontiguousarray(
        Wq.reshape(DC, P, DC, P).transpose(2, 1, 0, 3)).astype(np_xw)
    wk_h = np.ascontiguousarray(
        Wk.reshape(DC, P, DC, P).transpose(2, 1, 0, 3)).astype(np_xw)
    wv_h = np.ascontiguousarray(
        Wv.reshape(2, 8, P, 4, 512).transpose(3, 0, 2, 1, 4)).astype(np_xw)

    k_in_block = np.arange(P, dtype=np.int64)[:, None]           # [P, 1]
    q_in_chunk = np.arange(512, dtype=np.int64)[None, :]         # [1, 512]

    def build_masks(h):
        c_lo, c_hi = h, 3 - h
        masks = np.zeros((P, 24, 512), dtype=ml_dtypes.bfloat16)
        for j, pos in enumerate(SLOT0_POS):
            tkb = POS2TRUE[pos // 4] * 4 + pos % 4
            masks[:, j, :] = (tkb * P + k_in_block) <= (c_lo * 512 + q_in_chunk)
        for pos in range(SB):
            tkb = POS2TRUE[pos // 4] * 4 + pos % 4
            masks[:, 8 + pos, :] = (tkb * P + k_in_block) <= (c_hi * 512 + q_in_chunk)
        return masks

    mask_h = [build_masks(0), build_masks(1)]

    in_maps = []
    for core in range(NCORES):
        b, h = divmod(core, 2)
        c_lo, c_hi = h, 3 - h
        xt = x[b].T                                               # [D, S] view
        xtp = np.concatenate(
            [xt[:, c_lo * 512:(c_lo + 1) * 512],
             xt[:, c_hi * 512:(c_hi + 1) * 512]], axis=1)         # [D, 1024]
        in_maps.append({
            "xt": np.ascontiguousarray(xtp).astype(np_xw),
            "wq": wq_h, "wk": wk_h, "wv": wv_h, "masks": mask_h[h],
        })
    return in_maps


def run(x, Wq, Wk, Wv, trace=False):
    x = np.asarray(x, dtype=np.float32)
    Wq = np.asarray(Wq, dtype=np.float32)
    Wk = np.asarray(Wk, dtype=np.float32)
    Wv = np.asarray(Wv, dtype=np.float32)
    nc = build_nc()
    in_maps = _host_prep(x, Wq, Wk, Wv)
    res = run_bass_kernel_spmd(nc, in_maps, core_ids=list(range(NCORES)),
                               trace=trace)
    out = np.empty((B, S, D), dtype=np.float32)
    for core in range(NCORES):
        b, h = divmod(core, 2)
        c_lo, c_hi = h, 3 - h
        o = res.results[core]["out"]
        out[b, c_lo * 512:(c_lo + 1) * 512] = o[:512]
        out[b, c_hi * 512:(c_hi + 1) * 512] = o[512:]
    return out, res


def kernel(x, Wq, Wk, Wv):
    out, _ = run(x, Wq, Wk, Wv)
    return out


if __name__ == "__main__":
    build_nc()
    print("build + compile OK")


# revision 19
# speedup vs baseline: 1.0363x; 1.0363x over previous
"""Causal attention kernel for Trainium2, 8 NeuronCores.

Problem: x[4,2048,2048] @ Wq/Wk/Wv[2048,2048] -> causal softmax attention.

Sharding: 2 cores per batch; each core owns 1024 query rows, assigned as
global 512-row chunks {0,3} (even cores) / {1,2} (odd cores) to balance causal
work. Each core computes Q^T, K^T and V only for its OWN rows (its x^T input
holds just those 1024 columns), then a pairwise AllGather assembles the full
K^T / V for the batch. Gathered key blocks land in pair-rank order
[chunk0, chunk3, chunk1, chunk2], identical on every core, so the SPMD
program is uniform: query slot 0 (chunk c_lo) attends to key positions
{0-3, 8-11}, slot 1 (chunk c_hi) to all 16; true causality for the permuted
key order is enforced by per-core mask tensors (input data).

On-device layout: scores are computed transposed (S^T = K Q^T, keys on
partitions) so exp(S^T) feeds the attention@V matmul directly as the
stationary operand with no transpose; row sums come from a matmul against a
ones vector; softmax max-subtraction is skipped (softmax is shift-invariant;
scores are O(1) here so fp32 range is ample).

dtypes: projections + scores in float32r (full PE rate, ~1e-4 rounding),
probabilities/V in bf16 for the AV matmul, all accumulation fp32.
"""

import math

import numpy as np
import ml_dtypes

import concourse.bass as bass
import concourse.mybir as mybir
import concourse.tile as tile
from concourse import bacc
from concourse.bass import ds, ts
from concourse.bass_utils import run_bass_kernel_spmd

B, S, D = 4, 2048, 2048
P = 128
DC = D // P          # 16 contraction chunks
SB = S // P          # 16 key blocks
QROWS = 1024         # query rows per core
NCORES = 8
INV_SQRT_D = 1.0 / math.sqrt(D)

# gathered key-block position -> true 512-chunk (pair-rank order, all cores)
POS2TRUE = [0, 3, 1, 2]
# key-block positions processed by query slot 0
SLOT0_POS = [0, 1, 2, 3, 8, 9, 10, 11]
S0IDX = {pos: j for j, pos in enumerate(SLOT0_POS)}
PAIRS = [[0, 1], [2, 3], [4, 5], [6, 7]]

F32 = mybir.dt.float32
F32R = mybir.dt.float32r
BF16 = mybir.dt.bfloat16
XW_DT = BF16    # x^T and W inputs + projection matmuls
QK_DT = BF16    # Q^T/K^T staging + score matmuls
Exp = mybir.ActivationFunctionType.Exp

_CACHED_NC = None


def build_nc():
    global _CACHED_NC
    if _CACHED_NC is not None:
        return _CACHED_NC
    nc = bacc.Bacc(trn_type="TRN2", target_bir_lowering=False, debug=False,
                   num_devices=NCORES)

    xt_d = nc.dram_tensor("xt", [D, QROWS], XW_DT, kind="ExternalInput")
    wq_d = nc.dram_tensor("wq", [DC, P, DC, P], XW_DT, kind="ExternalInput")
    wk_d = nc.dram_tensor("wk", [DC, P, DC, P], XW_DT, kind="ExternalInput")
    wv_d = nc.dram_tensor("wv", [4, 2, P, 8, 512], XW_DT, kind="ExternalInput")
    mk_d = nc.dram_tensor("masks", [P, 24, 512], BF16, kind="ExternalInput")
    out_d = nc.dram_tensor("out", [QROWS, D], F32, kind="ExternalOutput")

    with tile.TileContext(nc) as tc:
        with tc.tile_pool(name="dram", bufs=1, space="DRAM") as dpool:
            qT = dpool.tile([P, DC, QROWS], QK_DT, tag="qT")      # [p, dc, q]
            kT_own = dpool.tile([8, P, DC, P], QK_DT, tag="kTo")  # [kb, p, dc, k]
            vv_own = dpool.tile([8, P, D], BF16, tag="vvo")      # [kb, p, d]
            # gathered, chunked so each AllGather can overlap compute
            kgs = [dpool.tile([2, 4, P, DC, P], QK_DT, tag=f"kg{g}",
                              name=f"kg{g}") for g in range(2)]
            vgs = [dpool.tile([2, 2, P, D], BF16, tag=f"vg{g}",
                              name=f"vg{g}") for g in range(4)]

            # ---------------- phase 1: projections ----------------
            with (
                tc.tile_pool(name="xt", bufs=1) as xt_pool,
                tc.tile_pool(name="wqk", bufs=16) as wqk_pool,
                tc.tile_pool(name="st", bufs=6) as st_pool,
                tc.tile_pool(name="ps1", bufs=8, space="PSUM") as ps1,
            ):
                # whole-phase W preloads (bf16 W is small enough to keep
                # an entire projection's weights resident); first few W tiles
                # and x^T go first so the K matmuls can start early
                def load_w(pool, dram, m, name):
                    wt = pool.tile([P, DC, P], XW_DT, tag="w", name=name)
                    nc.sync.dma_start(wt[:, :8, :], dram.ap()[m][:, :8, :])
                    nc.sync.dma_start(wt[:, 8:, :], dram.ap()[m][:, 8:, :])
                    return wt

                wk_pre = [load_w(wqk_pool, wk_d, m, f"wkp{m}") for m in range(4)]

                xts = [xt_pool.tile([P, DC, 512], XW_DT, tag=f"xt{c}",
                                    name=f"xt{c}")
                       for c in range(2)]
                for c in range(2):
                    for dc in range(DC):
                        nc.sync.dma_start(
                            xts[c][:, dc, :],
                            xt_d.ap()[ds(dc * P, P), ts(c, 512)])
                wk_pre += [load_w(wqk_pool, wk_d, m, f"wkp{m}")
                           for m in range(4, DC)]

                # --- K^T (own rows only), s-outer so each 4-block half
                # can be gathered while the rest of phase 1 computes
                for s in range(2):
                    for m in range(DC):
                        wt = wk_pre[m]
                        ps = ps1.tile([P, 512], F32, tag="ps")
                        for dc in range(DC):
                            nc.tensor.matmul(
                                ps[:], lhsT=wt[:, dc, :],
                                rhs=xts[s][:, dc, :],
                                start=(dc == 0), stop=(dc == DC - 1),
                            )
                        st = st_pool.tile([P, 512], QK_DT, tag="st")
                        nc.scalar.copy(st[:], ps[:])
                        for j in range(4):
                            nc.sync.dma_start(kT_own[s * 4 + j, :, m, :],
                                              st[:, ts(j, P)])
                    nc.gpsimd.collective_compute(
                        "AllGather", mybir.AluOpType.bypass,
                        replica_groups=PAIRS,
                        ins=[kT_own[ds(s * 4, 4)].opt()],
                        outs=[kgs[s].opt()],
                    )

                # wq preloads now: they fill the freed wk slots during V,
                # so the Q phase needs no DMA while the gathers run
                wq_pre = [load_w(wqk_pool, wq_d, m, f"wq{m}")
                          for m in range(DC)]

                # --- V (own rows only, natural layout, bf16)
                with (
                    tc.tile_pool(name="wv", bufs=8) as wv_pool,
                    tc.tile_pool(name="stv", bufs=4) as stv_pool,
                ):
                    wv_pre = []
                    for n in range(4):
                        for hb in range(2):
                            wvt = wv_pool.tile([P, 8, 512], XW_DT, tag="wv",
                                               name=f"wv{n}{hb}")
                            nc.sync.dma_start(wvt[:, :4, :],
                                              wv_d.ap()[n, hb][:, :4, :])
                            nc.sync.dma_start(wvt[:, 4:, :],
                                              wv_d.ap()[n, hb][:, 4:, :])
                            wv_pre.append(wvt)
                    for s in range(8):
                        for n in range(4):
                            wva = wv_pre[2 * n]
                            wvb = wv_pre[2 * n + 1]
                            ps = ps1.tile([P, 512], F32, tag="ps")
                            for dc in range(DC):
                                w = wva if dc < 8 else wvb
                                nc.tensor.matmul(
                                    ps[:], lhsT=xts[s // 4][:, dc, ts(s % 4, P)],
                                    rhs=w[:, dc % 8, :],
                                    start=(dc == 0), stop=(dc == DC - 1),
                                )
                            sv = stv_pool.tile([P, 512], BF16, tag="sv")
                            nc.vector.tensor_copy(sv[:], ps[:])
                            nc.sync.dma_start(vv_own[s, :, ts(n, 512)], sv[:])
                        if s % 2 == 1:
                            g = s // 2
                            nc.gpsimd.collective_compute(
                                "AllGather", mybir.AluOpType.bypass,
                                replica_groups=PAIRS,
                                ins=[vv_own[ds(g * 2, 2)].opt()],
                                outs=[vgs[g].opt()],
                            )

                # --- Q^T (no DMA besides qT writes: wq already resident)
                for m in range(DC):
                    wt = wq_pre[m]
                    for s in range(2):
                        ps = ps1.tile([P, 512], F32, tag="ps")
                        for dc in range(DC):
                            nc.tensor.matmul(
                                ps[:], lhsT=wt[:, dc, :],
                                rhs=xts[s][:, dc, :],
                                start=(dc == 0), stop=(dc == DC - 1),
                            )
                        st = st_pool.tile([P, 512], QK_DT, tag="st")
                        nc.scalar.copy(st[:], ps[:])
                        nc.sync.dma_start(qT[:, m, ts(s, 512)], st[:])

            # ---------------- phase 2: attention ----------------
            with (
                tc.tile_pool(name="qtv", bufs=2) as qtv_pool,
                tc.tile_pool(name="kt2", bufs=12) as kt_pool,
                tc.tile_pool(name="pt", bufs=1) as pt_pool,
                tc.tile_pool(name="mk", bufs=1) as mk_pool,
                tc.tile_pool(name="one", bufs=1) as one_pool,
                tc.tile_pool(name="sc", bufs=4) as sc_pool,
                tc.tile_pool(name="ob", bufs=4) as ob_pool,
                tc.tile_pool(name="ps_s", bufs=4, space="PSUM") as ps_s,
                tc.tile_pool(name="ps_av", bufs=3, space="PSUM") as ps_av,
                tc.tile_pool(name="ps_l", bufs=1, space="PSUM") as ps_l,
            ):
                mk = mk_pool.tile([P, 24, 512], BF16, tag="mk")
                for j in range(3):
                    nc.sync.dma_start(mk[:, ts(j, 8), :], mk_d.ap()[:, ts(j, 8), :])
                ones = one_pool.tile([P, 1], BF16, tag="ones")
                nc.vector.memset(ones[:], 1.0)
                # pt index: slot0 j -> key pos SLOT0_POS[j]; slot1 kb -> 8+kb
                pt = pt_pool.tile([P, 24, 512], BF16, tag="pt")

                # --- scores + exp.  qt/vt share pool slots (tag "qt"): the
                # two V super-tiles allocate into the slots the q tiles
                # release after their last score matmul.
                qts = []
                for slot in range(2):
                    qt = qtv_pool.tile([P, DC, 512], QK_DT, tag="qt",
                                       name=f"qt{slot}")
                    for j in range(4):
                        nc.sync.dma_start(qt[:, ts(j, 4), :],
                                          qT[:, ts(j, 4), ts(slot, 512)])
                    qts.append(qt)

                for kb in range(SB):
                    kt_t = kt_pool.tile([P, DC, P], QK_DT, tag="kt",
                                        name=f"kt{kb}")
                    o = kb % 8
                    src = kgs[o // 4][kb // 8, o % 4]
                    for j in range(4):
                        nc.sync.dma_start(kt_t[:, ts(j, 4), :],
                                          src[:, ts(j, 4), :])
                    targets = []
                    if kb in S0IDX:
                        targets.append((S0IDX[kb], 0))
                    targets.append((8 + kb, 1))
                    for pti, slot in targets:
                        ps = ps_s.tile([P, 512], F32, tag="ps")
                        for dc in range(DC):
                            nc.tensor.matmul(
                                ps[:], lhsT=kt_t[:, dc, :],
                                rhs=qts[slot][:, dc, :],
                                start=(dc == 0), stop=(dc == DC - 1),
                            )
                        nc.scalar.activation(pt[:, pti, :], ps[:], Exp,
                                             scale=INV_SQRT_D)

                # --- V tiles reuse the qt slots; masks; AV; normalize
                vbig = []
                for half in range(2):
                    vb = qtv_pool.tile([P, 8, D], BF16, tag="qt",
                                       name=f"vb{half}")
                    for j in range(8):
                        nc.sync.dma_start(vb[:, j, :], vgs[j // 2][half, j % 2])
                    vbig.append(vb)

                def vt_ap(pos):
                    return vbig[pos // 8][:, pos % 8, :]

                for j in range(24):
                    nc.vector.tensor_mul(pt[:, j, :], pt[:, j, :], mk[:, j, :])

                for slot in range(2):
                    if slot == 0:
                        idx = list(range(8))
                        kpos = SLOT0_POS
                    else:
                        idx = list(range(8, 24))
                        kpos = list(range(SB))
                    for qs in range(4):
                        pl = ps_l.tile([P, 1], F32, tag="pl")
                        for i, j in enumerate(idx):
                            nc.tensor.matmul(
                                pl[:], lhsT=pt[:, j, ts(qs, P)],
                                rhs=ones[:],
                                start=(i == 0), stop=(i == len(idx) - 1),
                            )
                        rl = sc_pool.tile([P, 1], F32, tag="rl")
                        nc.vector.reciprocal(rl[:], pl[:])
                        for n in range(4):
                            pav = ps_av.tile([P, 512], F32, tag="pav")
                            for i, j in enumerate(idx):
                                nc.tensor.matmul(
                                    pav[:], lhsT=pt[:, j, ts(qs, P)],
                                    rhs=vt_ap(kpos[i])[:, ts(n, 512)],
                                    start=(i == 0),
                                    stop=(i == len(idx) - 1),
                                )
                            ob = ob_pool.tile([P, 512], F32, tag="ob")
                            nc.vector.tensor_scalar_mul(ob[:], pav[:], rl[:])
                            nc.sync.dma_start(
                                out_d.ap()[ds(slot * 512 + qs * P, P),
                                           ts(n, 512)],
                                ob[:],
                            )

    nc.compile()
    _CACHED_NC = nc
    return nc


def _host_prep(x, Wq, Wk, Wv):
    """Build per-core input maps (host-side layout prep)."""
    np_xw = ml_dtypes.bfloat16 if XW_DT == BF16 else np.float32
    wq_h = np.ascontiguousarray(
        Wq.reshape(DC, P, DC, P).transpose(2, 1, 0, 3)).astype(np_xw)
    wk_h = np.ascontiguousarray(
        Wk.reshape(DC, P, DC, P).transpose(2, 1, 0, 3)).astype(np_xw)
    wv_h = np.ascontiguousarray(
        Wv.reshape(2, 8, P, 4, 512).transpose(3, 0, 2, 1, 4)).astype(np_xw)

    k_in_block = np.arange(P, dtype=np.int64)[:, None]           # [P, 1]
    q_in_chunk = np.arange(512, dtype=np.int64)[None, :]         # [1, 512]

    def build_masks(h):
        c_lo, c_hi = h, 3 - h
        masks = np.zeros((P, 24, 512), dtype=ml_dtypes.bfloat16)
        for j, pos in enumerate(SLOT0_POS):
            tkb = POS2TRUE[pos // 4] * 4 + pos % 4
            masks[:, j, :] = (tkb * P + k_in_block) <= (c_lo * 512 + q_in_chunk)
        for pos in range(SB):
            tkb = POS2TRUE[pos // 4] * 4 + pos % 4
            masks[:, 8 + pos, :] = (tkb * P + k_in_block) <= (c_hi * 512 + q_in_chunk)
        return masks

    mask_h = [build_masks(0), build_masks(1)]

    in_maps = []
    for core in range(NCORES):
        b, h = divmod(core, 2)
        c_lo, c_hi = h, 3 - h
        xt = x[b].T                                               # [D, S] view
        xtp = np.concatenate(
            [xt[:, c_lo * 512:(c_lo + 1) * 512],
             xt[:, c_hi * 512:(c_hi + 1) * 512]], axis=1)         # [D, 1024]
        in_maps.append({
            "xt": np.ascontiguousarray(xtp).astype(np_xw),
            "wq": wq_h, "wk": wk_h, "wv": wv_h, "masks": mask_h[h],
        })
    return in_maps


def run(x, Wq, Wk, Wv, trace=False):
    x = np.asarray(x, dtype=np.float32)
    Wq = np.asarray(Wq, dtype=np.float32)
    Wk = np.asarray(Wk, dtype=np.float32)
    Wv = np.asarray(Wv, dtype=np.float32)
    nc = build_nc()
    in_maps = _host_prep(x, Wq, Wk, Wv)
    res = run_bass_kernel_spmd(nc, in_maps, core_ids=list(range(NCORES)),
                               trace=trace)
    out = np.empty((B, S, D), dtype=np.float32)
    for core in range(NCORES):
        b, h = divmod(core, 2)
        c_lo, c_hi = h, 3 - h
        o = res.results[core]["out"]
        out[b, c_lo * 512:(c_lo + 1) * 512] = o[:512]
        out[b, c_hi * 512:(c_hi + 1) * 512] = o[512:]
    return out, res


def kernel(x, Wq, Wk, Wv):
    out, _ = run(x, Wq, Wk, Wv)
    return out


if __name__ == "__main__":
    build_nc()
    print("build + compile OK")


# revision 20
# speedup vs baseline: 1.1972x; 1.1553x over previous
"""Causal attention kernel for Trainium2, 8 NeuronCores.

Problem: x[4,2048,2048] @ Wq/Wk/Wv[2048,2048] -> causal softmax attention.

Sharding: 2 cores per batch; each core owns 1024 query rows, assigned as
global 512-row chunks {0,3} (even cores) / {1,2} (odd cores) to balance causal
work. Each core computes Q^T, K^T and V only for its OWN rows (its x^T input
holds just those 1024 columns), then a pairwise AllGather assembles the full
K^T / V for the batch. Gathered key blocks land in pair-rank order
[chunk0, chunk3, chunk1, chunk2], identical on every core, so the SPMD
program is uniform: query slot 0 (chunk c_lo) attends to key positions
{0-3, 8-11}, slot 1 (chunk c_hi) to all 16; true causality for the permuted
key order is enforced by per-core mask tensors (input data).

On-device layout: scores are computed transposed (S^T = K Q^T, keys on
partitions) so exp(S^T) feeds the attention@V matmul directly as the
stationary operand with no transpose; row sums come from a matmul against a
ones vector; softmax max-subtraction is skipped (softmax is shift-invariant;
scores are O(1) here so fp32 range is ample).

dtypes: projections + scores in float32r (full PE rate, ~1e-4 rounding),
probabilities/V in bf16 for the AV matmul, all accumulation fp32.
"""

import math

import numpy as np
import ml_dtypes

import concourse.bass as bass
import concourse.mybir as mybir
import concourse.tile as tile
from concourse import bacc
from concourse.bass import ds, ts
from concourse.bass_utils import run_bass_kernel_spmd

B, S, D = 4, 2048, 2048
P = 128
DC = D // P          # 16 contraction chunks
SB = S // P          # 16 key blocks
QROWS = 1024         # query rows per core
NCORES = 8
INV_SQRT_D = 1.0 / math.sqrt(D)

# gathered key-block position -> true 512-chunk (pair-rank order, all cores)
POS2TRUE = [0, 3, 1, 2]
# key-block positions processed by query slot 0
SLOT0_POS = [0, 1, 2, 3, 8, 9, 10, 11]
S0IDX = {pos: j for j, pos in enumerate(SLOT0_POS)}
PAIRS = [[0, 1], [2, 3], [4, 5], [6, 7]]

F32 = mybir.dt.float32
F32R = mybir.dt.float32r
BF16 = mybir.dt.bfloat16
XW_DT = BF16    # x^T and W inputs + projection matmuls
QK_DT = BF16    # Q^T/K^T staging + score matmuls
Exp = mybir.ActivationFunctionType.Exp

_CACHED_NC = None


def build_nc():
    global _CACHED_NC
    if _CACHED_NC is not None:
        return _CACHED_NC
    nc = bacc.Bacc(trn_type="TRN2", target_bir_lowering=False, debug=False,
                   num_devices=NCORES)

    xt_d = nc.dram_tensor("xt", [D, QROWS], XW_DT, kind="ExternalInput")
    wq_d = nc.dram_tensor("wq", [DC, P, DC, P], XW_DT, kind="ExternalInput")
    wk_d = nc.dram_tensor("wk", [DC, P, DC, P], XW_DT, kind="ExternalInput")
    wv_d = nc.dram_tensor("wv", [4, 2, P, 8, 512], XW_DT, kind="ExternalInput")
    mk_d = nc.dram_tensor("masks", [P, 24, 512], BF16, kind="ExternalInput")
    out_d = nc.dram_tensor("out", [QROWS, D], F32, kind="ExternalOutput")

    with tile.TileContext(nc) as tc:
        with tc.tile_pool(name="dram", bufs=1, space="DRAM") as dpool:
            qT = dpool.tile([P, DC, QROWS], QK_DT, tag="qT")      # [p, dc, q]
            # per-gather-chunk tiles (separate so a chunk's AllGather read
            # never false-conflicts with later chunk writes)
            kT_own = [dpool.tile([4, P, DC, P], QK_DT, tag=f"kTo{s}",
                                 name=f"kTo{s}") for s in range(2)]
            vv_own = [dpool.tile([2, P, D], BF16, tag=f"vvo{g}",
                                 name=f"vvo{g}") for g in range(4)]
            # gathered, chunked so each AllGather can overlap compute
            kgs = [dpool.tile([2, 4, P, DC, P], QK_DT, tag=f"kg{g}",
                              name=f"kg{g}") for g in range(2)]
            vgs = [dpool.tile([2, 2, P, D], BF16, tag=f"vg{g}",
                              name=f"vg{g}") for g in range(4)]

            # ---------------- phase 1: projections ----------------
            with (
                tc.tile_pool(name="xt", bufs=1) as xt_pool,
                tc.tile_pool(name="wqk", bufs=16) as wqk_pool,
                tc.tile_pool(name="st", bufs=6) as st_pool,
                tc.tile_pool(name="ps1", bufs=8, space="PSUM") as ps1,
            ):
                # whole-phase W preloads (bf16 W is small enough to keep
                # an entire projection's weights resident); first few W tiles
                # and x^T go first so the K matmuls can start early
                def load_w(pool, dram, m, name):
                    wt = pool.tile([P, DC, P], XW_DT, tag="w", name=name)
                    nc.sync.dma_start(wt[:, :8, :], dram.ap()[m][:, :8, :])
                    nc.sync.dma_start(wt[:, 8:, :], dram.ap()[m][:, 8:, :])
                    return wt

                wk_pre = [load_w(wqk_pool, wk_d, m, f"wkp{m}") for m in range(4)]

                xts = [xt_pool.tile([P, DC, 512], XW_DT, tag=f"xt{c}",
                                    name=f"xt{c}")
                       for c in range(2)]
                for c in range(2):
                    for dc in range(DC):
                        nc.sync.dma_start(
                            xts[c][:, dc, :],
                            xt_d.ap()[ds(dc * P, P), ts(c, 512)])
                wk_pre += [load_w(wqk_pool, wk_d, m, f"wkp{m}")
                           for m in range(4, DC)]

                # --- K^T (own rows only), s-outer so each 4-block half
                # can be gathered while the rest of phase 1 computes
                for s in range(2):
                    for m in range(DC):
                        wt = wk_pre[m]
                        ps = ps1.tile([P, 512], F32, tag="ps")
                        for dc in range(DC):
                            nc.tensor.matmul(
                                ps[:], lhsT=wt[:, dc, :],
                                rhs=xts[s][:, dc, :],
                                start=(dc == 0), stop=(dc == DC - 1),
                            )
                        st = st_pool.tile([P, 512], QK_DT, tag="st")
                        nc.scalar.copy(st[:], ps[:])
                        for j in range(4):
                            nc.gpsimd.dma_start(kT_own[s][j, :, m, :],
                                                st[:, ts(j, P)])
                    nc.gpsimd.collective_compute(
                        "AllGather", mybir.AluOpType.bypass,
                        replica_groups=PAIRS,
                        ins=[kT_own[s].opt()],
                        outs=[kgs[s].opt()],
                    )

                # wq preloads now: they fill the freed wk slots during V,
                # so the Q phase needs no DMA while the gathers run
                wq_pre = [load_w(wqk_pool, wq_d, m, f"wq{m}")
                          for m in range(DC)]

                # --- V (own rows only, natural layout, bf16)
                with (
                    tc.tile_pool(name="wv", bufs=8) as wv_pool,
                    tc.tile_pool(name="stv", bufs=4) as stv_pool,
                ):
                    wv_pre = []
                    for n in range(4):
                        for hb in range(2):
                            wvt = wv_pool.tile([P, 8, 512], XW_DT, tag="wv",
                                               name=f"wv{n}{hb}")
                            nc.sync.dma_start(wvt[:, :4, :],
                                              wv_d.ap()[n, hb][:, :4, :])
                            nc.sync.dma_start(wvt[:, 4:, :],
                                              wv_d.ap()[n, hb][:, 4:, :])
                            wv_pre.append(wvt)
                    for s in range(8):
                        for n in range(4):
                            wva = wv_pre[2 * n]
                            wvb = wv_pre[2 * n + 1]
                            ps = ps1.tile([P, 512], F32, tag="ps")
                            for dc in range(DC):
                                w = wva if dc < 8 else wvb
                                nc.tensor.matmul(
                                    ps[:], lhsT=xts[s // 4][:, dc, ts(s % 4, P)],
                                    rhs=w[:, dc % 8, :],
                                    start=(dc == 0), stop=(dc == DC - 1),
                                )
                            sv = stv_pool.tile([P, 512], BF16, tag="sv")
                            nc.vector.tensor_copy(sv[:], ps[:])
                            nc.gpsimd.dma_start(
                                vv_own[s // 2][s % 2, :, ts(n, 512)], sv[:])
                        if s % 2 == 1:
                            g = s // 2
                            nc.gpsimd.collective_compute(
                                "AllGather", mybir.AluOpType.bypass,
                                replica_groups=PAIRS,
                                ins=[vv_own[g].opt()],
                                outs=[vgs[g].opt()],
                            )

                # --- Q^T (no DMA besides qT writes: wq already resident)
                for m in range(DC):
                    wt = wq_pre[m]
                    for s in range(2):
                        ps = ps1.tile([P, 512], F32, tag="ps")
                        for dc in range(DC):
                            nc.tensor.matmul(
                                ps[:], lhsT=wt[:, dc, :],
                                rhs=xts[s][:, dc, :],
                                start=(dc == 0), stop=(dc == DC - 1),
                            )
                        st = st_pool.tile([P, 512], QK_DT, tag="st")
                        nc.scalar.copy(st[:], ps[:])
                        nc.gpsimd.dma_start(qT[:, m, ts(s, 512)], st[:])

            # ---------------- phase 2: attention ----------------
            with (
                tc.tile_pool(name="qtv", bufs=2) as qtv_pool,
                tc.tile_pool(name="kt2", bufs=12) as kt_pool,
                tc.tile_pool(name="pt", bufs=1) as pt_pool,
                tc.tile_pool(name="mk", bufs=1) as mk_pool,
                tc.tile_pool(name="one", bufs=1) as one_pool,
                tc.tile_pool(name="sc", bufs=4) as sc_pool,
                tc.tile_pool(name="ob", bufs=4) as ob_pool,
                tc.tile_pool(name="ps_s", bufs=4, space="PSUM") as ps_s,
                tc.tile_pool(name="ps_av", bufs=3, space="PSUM") as ps_av,
                tc.tile_pool(name="ps_l", bufs=1, space="PSUM") as ps_l,
            ):
                mk = mk_pool.tile([P, 24, 512], BF16, tag="mk")
                for j in range(3):
                    nc.sync.dma_start(mk[:, ts(j, 8), :], mk_d.ap()[:, ts(j, 8), :])
                ones = one_pool.tile([P, 1], BF16, tag="ones")
                nc.vector.memset(ones[:], 1.0)
                # pt index: slot0 j -> key pos SLOT0_POS[j]; slot1 kb -> 8+kb
                pt = pt_pool.tile([P, 24, 512], BF16, tag="pt")

                # --- scores + exp.  qt/vt share pool slots (tag "qt"): the
                # two V super-tiles allocate into the slots the q tiles
                # release after their last score matmul.
                qts = []
                for slot in range(2):
                    qt = qtv_pool.tile([P, DC, 512], QK_DT, tag="qt",
                                       name=f"qt{slot}")
                    for j in range(4):
                        nc.sync.dma_start(qt[:, ts(j, 4), :],
                                          qT[:, ts(j, 4), ts(slot, 512)])
                    qts.append(qt)

                for kb in range(SB):
                    kt_t = kt_pool.tile([P, DC, P], QK_DT, tag="kt",
                                        name=f"kt{kb}")
                    o = kb % 8
                    src = kgs[o // 4][kb // 8, o % 4]
                    for j in range(4):
                        nc.sync.dma_start(kt_t[:, ts(j, 4), :],
                                          src[:, ts(j, 4), :])
                    targets = []
                    if kb in S0IDX:
                        targets.append((S0IDX[kb], 0))
                    targets.append((8 + kb, 1))
                    for pti, slot in targets:
                        ps = ps_s.tile([P, 512], F32, tag="ps")
                        for dc in range(DC):
                            nc.tensor.matmul(
                                ps[:], lhsT=kt_t[:, dc, :],
                                rhs=qts[slot][:, dc, :],
                                start=(dc == 0), stop=(dc == DC - 1),
                            )
                        nc.scalar.activation(pt[:, pti, :], ps[:], Exp,
                                             scale=INV_SQRT_D)

                # --- V tiles reuse the qt slots; masks; AV; normalize
                vbig = []
                for half in range(2):
                    vb = qtv_pool.tile([P, 8, D], BF16, tag="qt",
                                       name=f"vb{half}")
                    for j in range(8):
                        nc.sync.dma_start(vb[:, j, :], vgs[j // 2][half, j % 2])
                    vbig.append(vb)

                def vt_ap(pos):
                    return vbig[pos // 8][:, pos % 8, :]

                for j in range(24):
                    nc.vector.tensor_mul(pt[:, j, :], pt[:, j, :], mk[:, j, :])

                for slot in range(2):
                    if slot == 0:
                        idx = list(range(8))
                        kpos = SLOT0_POS
                    else:
                        idx = list(range(8, 24))
                        kpos = list(range(SB))
                    for qs in range(4):
                        pl = ps_l.tile([P, 1], F32, tag="pl")
                        for i, j in enumerate(idx):
                            nc.tensor.matmul(
                                pl[:], lhsT=pt[:, j, ts(qs, P)],
                                rhs=ones[:],
                                start=(i == 0), stop=(i == len(idx) - 1),
                            )
                        rl = sc_pool.tile([P, 1], F32, tag="rl")
                        nc.vector.reciprocal(rl[:], pl[:])
                        for n in range(4):
                            pav = ps_av.tile([P, 512], F32, tag="pav")
                            for i, j in enumerate(idx):
                                nc.tensor.matmul(
                                    pav[:], lhsT=pt[:, j, ts(qs, P)],
                                    rhs=vt_ap(kpos[i])[:, ts(n, 512)],
                                    start=(i == 0),
                                    stop=(i == len(idx) - 1),
                                )
                            ob = ob_pool.tile([P, 512], F32, tag="ob")
                            nc.vector.tensor_scalar_mul(ob[:], pav[:], rl[:])
                            nc.sync.dma_start(
                                out_d.ap()[ds(slot * 512 + qs * P, P),
                                           ts(n, 512)],
                                ob[:],
                            )

    nc.compile()
    _CACHED_NC = nc
    return nc


def _host_prep(x, Wq, Wk, Wv):
    """Build per-core input maps (host-side layout prep)."""
    np_xw = ml_dtypes.bfloat16 if XW_DT == BF16 else np.float32
    wq_h = np.ascontiguousarray(
        Wq.reshape(DC, P, DC, P).transpose(2, 1, 0, 3)).astype(np_xw)
    wk_h = np.ascontiguousarray(
        Wk.reshape(DC, P, DC, P).transpose(2, 1, 0, 3)).astype(np_xw)
    wv_h = np.ascontiguousarray(
        Wv.reshape(2, 8, P, 4, 512).transpose(3, 0, 2, 1, 4)).astype(np_xw)

    k_in_block = np.arange(P, dtype=np.int64)[:, None]           # [P, 1]
    q_in_chunk = np.arange(512, dtype=np.int64)[None, :]         # [1, 512]

    def build_masks(h):
        c_lo, c_hi = h, 3 - h
        masks = np.zeros((P, 24, 512), dtype=ml_dtypes.bfloat16)
        for j, pos in enumerate(SLOT0_POS):
            tkb = POS2TRUE[pos // 4] * 4 + pos % 4
            masks[:, j, :] = (tkb * P + k_in_block) <= (c_lo * 512 + q_in_chunk)
        for pos in range(SB):
            tkb = POS2TRUE[pos // 4] * 4 + pos % 4
            masks[:, 8 + pos, :] = (tkb * P + k_in_block) <= (c_hi * 512 + q_in_chunk)
        return masks

    mask_h = [build_masks(0), build_masks(1)]

    in_maps = []
    for core in range(NCORES):
        b, h = divmod(core, 2)
        c_lo, c_hi = h, 3 - h
        xt = x[b].T                                               # [D, S] view
        xtp = np.concatenate(
            [xt[:, c_lo * 512:(c_lo + 1) * 512],
             xt[:, c_hi * 512:(c_hi + 1) * 512]], axis=1)         # [D, 1024]
        in_maps.append({
            "xt": np.ascontiguousarray(xtp).astype(np_xw),
            "wq": wq_h, "wk": wk_h, "wv": wv_h, "masks": mask_h[h],
        })
    return in_maps


def run(x, Wq, Wk, Wv, trace=False):
    x = np.asarray(x, dtype=np.float32)
    Wq = np.asarray(Wq, dtype=np.float32)
    Wk = np.asarray(Wk, dtype=np.float32)
    Wv = np.asarray(Wv, dtype=np.float32)
    nc = build_nc()
    in_maps = _host_prep(x, Wq, Wk, Wv)
    res = run_bass_kernel_spmd(nc, in_maps, core_ids=list(range(NCORES)),
                               trace=trace)
    out = np.empty((B, S, D), dtype=np.float32)
    for core in range(NCORES):
        b, h = divmod(core, 2)
        c_lo, c_hi = h, 3 - h
        o = res.results[core]["out"]
        out[b, c_lo * 512:(c_lo + 1) * 512] = o[:512]
        out[b, c_hi * 512:(c_hi + 1) * 512] = o[512:]
    return out, res


def kernel(x, Wq, Wk, Wv):
    out, _ = run(x, Wq, Wk, Wv)
    return out


if __name__ == "__main__":
    build_nc()
    print("build + compile OK")
